# revision 24
# baseline (speedup 1.0000x reference)
"""Multi-head attention (B=2,S=2048,D=1024,H=16) on 8 TRN2 NeuronCores.

Sharding: core c handles head-PAIR c (heads 2c, 2c+1) of BOTH batches
(tensor parallel over heads; both batches per core so per-batch key-tile
counts need no SPMD padding). wq/wk/wv split column-wise by pair, wo
row-wise. Each core computes partial output projections [D,S] per batch;
the host sums the 8 partials, transposes, adds bo.

v2 design (vs v1 baseline):
  - vT computed like kT (wv chunks stationary, 512-wide moving) then
    PE-transposed per 128-tile into AV layout -> kills the 198 tiny
    LDW-bound matmuls of v1.
  - q/k/v biases folded into the PSUM->SBUF copies (tensor_scalar_add
    with per-partition bias column) -> no bias matmuls.
  - Fully interleaved emission: ScalarE exp (the 117us wall: 88 ACTs)
    starts ~10us in and streams continuously; all proj/outproj PE work
    is emitted in small "filler" units inside attention t-loops so the
    PE works during exp waits instead of front-loading projections.
  - Finer, need-ordered input DMAs; merged output DMAs.

Per-core device layout ("T" = [feature, seq]):
  qT[b] = (wq_p^T @ xq_b^T)*0.125 + bq/8   [128, S]
  kT[b] =  wk_p^T @ xk_b^T + bk            [128, S]  (valid cols only)
  vT[b] =  wv_p^T @ xv_b^T + bv            [128, S]  -> transpose 128-tiles
  v[b]  [sk, b, t, head, 65] with ones column for softmax denominators
  per (b, sq-block, key-tile t):
    scoresT(hh) = kT_h[:,t]^T-stat @ qT_h    [128 sk, 512 sq] psum (pair
      co-runs on PE row groups 0-63 / 64-127)
    pT = exp(scoresT + mask_bias[b][t])      one 1024-wide ACT
    oT_ext(hh) += [v_h[t] | 1]^T-stat @ pT(hh)   [65, 512] psum
  oT = oT_ext[0:64] * bcast(1/rowsum);  outT[b] += wo_p^T @ oT
"""

import sys

if "/opt/trn_rl_repo" not in sys.path:
    sys.path.insert(0, "/opt/trn_rl_repo")

from contextlib import ExitStack

import numpy as np
import ml_dtypes

from concourse import bass, bacc, mybir
from concourse import tile
from concourse.bass_utils import run_bass_kernel_spmd

BF16 = mybir.dt.bfloat16
F32 = mybir.dt.float32
npbf16 = ml_dtypes.bfloat16

B, S, D, H, DH = 2, 2048, 1024, 16, 64
NCORES = 8
PW = 2 * DH  # 128: head-pair width = per-core projection width
NKC = D // 128  # 8 contraction chunks for projections
NST = S // 128  # 16 key tiles
SQB = 512
NSQB = S // SQB  # 4
NDT = D // 128  # 8 output row-tiles
SCALE = 1.0 / 8.0  # 1/sqrt(DH)
MASK_BIAS = -30000.0


def build_nc(nblks) -> bass.Bass:
    nblk0, nblk1 = nblks
    nkbs = tuple(-(-nb * 128 // SQB) for nb in nblks)  # valid 512-col blocks
    nc = bacc.Bacc()

    x_d = []
    for b in range(B):
        x_d.append(
            tuple(
                nc.declare_dram_parameter(f"x{n}t{b}", [D, S], BF16, isOutput=False)
                for n in "qkv"
            )
        )
    wq_d = nc.declare_dram_parameter("wq", [128, NKC * PW], BF16, isOutput=False)
    wk_d = nc.declare_dram_parameter("wk", [128, NKC * PW], BF16, isOutput=False)
    wv_d = nc.declare_dram_parameter("wv", [128, NKC * PW], BF16, isOutput=False)
    wo_d = nc.declare_dram_parameter("wo", [128, D], BF16, isOutput=False)
    bias_d = nc.declare_dram_parameter("bias", [128, 3], F32, isOutput=False)
    mb_d = nc.declare_dram_parameter("mb", [128, B * NST], F32, isOutput=False)
    id_d = nc.declare_dram_parameter("ident", [128, 128], BF16, isOutput=False)
    out_d = nc.declare_dram_parameter("outt", [B * D, S], BF16, isOutput=True)
    outv = out_d.rearrange("(x p) s -> p x s", p=128)  # [128, B*NDT, S]

    Exp = mybir.ActivationFunctionType.Exp

    with tile.TileContext(nc) as tc, ExitStack() as ctx:
        cpool = ctx.enter_context(tc.tile_pool(name="consts", bufs=1))
        xpool = ctx.enter_context(tc.tile_pool(name="xin", bufs=4))
        qkpool = ctx.enter_context(tc.tile_pool(name="qk", bufs=1))
        vtpool = ctx.enter_context(tc.tile_pool(name="vt", bufs=1))
        vpool = ctx.enter_context(tc.tile_pool(name="vsb", bufs=1))
        opool = ctx.enter_context(tc.tile_pool(name="osb", bufs=1))
        ptpool = ctx.enter_context(tc.tile_pool(name="ptp", bufs=4))
        smpool = ctx.enter_context(tc.tile_pool(name="small", bufs=3))
        outpool = ctx.enter_context(tc.tile_pool(name="outsb", bufs=2))
        pp = ctx.enter_context(tc.tile_pool(name="pp", bufs=2, space="PSUM"))
        sc = ctx.enter_context(tc.tile_pool(name="sc", bufs=2, space="PSUM"))
        otpp = ctx.enter_context(tc.tile_pool(name="otp", bufs=2, space="PSUM"))

        # ---- constants / weights ----
        wq_sb = cpool.tile([128, NKC * PW], BF16, tag="wq")
        wk_sb = cpool.tile([128, NKC * PW], BF16, tag="wk")
        wv_sb = cpool.tile([128, NKC * PW], BF16, tag="wv")
        wo_sb = cpool.tile([128, D], BF16, tag="wo")
        bias_sb = cpool.tile([128, 3], F32, tag="bias")
        mb_sb = cpool.tile([128, B, NST], F32, tag="mb")
        id_sb = cpool.tile([128, 128], BF16, tag="ident")

        qt_sb = qkpool.tile([128, B, S], BF16, tag="qt")
        kt_sb = qkpool.tile([128, B, S], BF16, tag="kt")
        # v with a trailing ones column per head (partition reads must be
        # 32-aligned, so the denominator sits at psum partition 64):
        # [sk-part, b, tile, head, dh+1]
        v_sb = vpool.tile([128, B, NST, 2, DH + 1], BF16, tag="v")
        nc.gpsimd.memset(v_sb[:, :, :, :, DH : DH + 1], 1.0)
        ot_sb = opool.tile([128, B, S], BF16, tag="ot")
        ones64 = cpool.tile([1, DH], F32, tag="ones64")
        nc.gpsimd.memset(ones64[:], 1.0)

        def xalloc():
            xq = xpool.tile([128, NKC, S], BF16, tag="xt", name="xq")
            xk = xpool.tile([128, NKC, S], BF16, tag="xt", name="xk")
            xv = xpool.tile([128, NKC, S], BF16, tag="xt", name="xv")
            return xq, xk, xv

        def xdram(b):
            return [x_d[b][i].rearrange("(c p) s -> p c s", p=128) for i in range(3)]

        def load_x1(b, nblk, xq, xk, xv):
            """Background batch: few big dispatches, need-ordered."""
            L = nblk * 128
            drams = xdram(b)
            nc.sync.dma_start(out=xq[:, :, 0:1024], in_=drams[0][:, :, 0:1024])
            for t, dram in ((xk, drams[1]), (xv, drams[2])):
                nc.sync.dma_start(out=t[:, :, 0:L], in_=dram[:, :, 0:L])
            nc.sync.dma_start(out=xq[:, :, 1024:S], in_=drams[0][:, :, 1024:S])

        # ---- projection chains: contraction over 8 chunks, bias folded
        # into the DVE copy. Emitted as pieces (2 MMs each) so they can be
        # spread into attention loops as PE filler.
        def chain_units(dst, w_sb, xt, c0, c1, bcol):
            n = c1 - c0
            box = []

            def piece(i):
                if i == 0:
                    box.append(pp.tile([128, SQB], F32, tag="pp", name="chps"))
                ps = box[0]
                for c in range(2 * i, 2 * i + 2):
                    nc.tensor.matmul(
                        ps[:, 0:n],
                        w_sb[:, c * PW : (c + 1) * PW],
                        xt[:, c, c0:c1],
                        start=(c == 0),
                        stop=(c == NKC - 1),
                    )
                if i == 3:
                    nc.vector.tensor_scalar_add(dst[:, c0:c1], ps[:, 0:n], bcol)

            return [(lambda i=i: piece(i)) for i in range(4)]

        def q_units(b, xt, s):
            return chain_units(
                qt_sb[:, b, :], wq_sb, xt, s * SQB, (s + 1) * SQB, bias_sb[:, 0:1]
            )

        def k_units(b, xt, j, nblk):
            c1 = min((j + 1) * SQB, nblk * 128)
            return chain_units(kt_sb[:, b, :], wk_sb, xt, j * SQB, c1, bias_sb[:, 1:2])

        def vt_units(b, xt, j, nblk, vt):
            c1 = min((j + 1) * SQB, nblk * 128)
            return chain_units(vt, wv_sb, xt, j * SQB, c1, bias_sb[:, 2:3])

        def transp_unit(b, vt, t):
            def u():
                ps = pp.tile([128, 128], BF16, tag="pp", name="trps")
                nc.tensor.transpose(
                    ps[:], vt[:, t * 128 : (t + 1) * 128], id_sb[:]
                )
                nc.vector.tensor_copy(
                    v_sb[:, b, t, :, 0:DH],
                    ps.rearrange("p (h d) -> p h d", h=2),
                )

            return u

        # ---- fused partial output projection, one unit per 128-row tile;
        # DMA per 4-tile half.
        def outproj_units(b, s, tail=False):
            sq0 = s * SQB
            halves = []

            def mk(dt):
                def u():
                    if dt % 4 == 0:
                        halves.append(outpool.tile([128, 4, SQB], BF16, tag="osb", name="osb"))
                    osb = halves[-1]
                    pso = pp.tile([128, SQB], F32, tag="pp", name="pso")
                    nc.tensor.matmul(
                        pso[:],
                        wo_sb[:, dt * 128 : (dt + 1) * 128],
                        ot_sb[:, b, sq0 : sq0 + SQB],
                        start=True,
                        stop=True,
                    )
                    if tail and dt % 2 == 1:
                        nc.scalar.copy(osb[:, dt % 4, :], pso[:])
                    else:
                        nc.vector.tensor_copy(osb[:, dt % 4, :], pso[:])
                    if dt % 4 == 3:
                        h = dt // 4
                        nc.sync.dma_start(
                            out=outv[
                                :, b * NDT + 4 * h : b * NDT + 4 * h + 4,
                                sq0 : sq0 + SQB,
                            ],
                            in_=osb[:],
                        )

                return u

            return [mk(dt) for dt in range(NDT)]

        # ---- filler machinery: (deadline, unit) lists consumed inside
        # attention t-loops. deadline d => must be emitted by stepper call
        # at t-step d (before scores(d+1) / av(d)). Chains are atomic
        # (consecutive units) but may interleave with other pp users only
        # pairwise (pp bufs=2).
        def make_stepper(nsteps, sched, horizon=6):
            dls = sorted([u for u in sched if u[0] <= nsteps], key=lambda x: x[0])
            bgs = [u for u in sched if u[0] > nsteps]
            state = [0, 0]

            def stepper(t):
                while state[0] < len(dls) and dls[state[0]][0] <= t:
                    dls[state[0]][1]()
                    state[0] += 1
                left = (len(dls) - state[0]) + (len(bgs) - state[1])
                k = -(-left // max(1, nsteps - t))
                while k > 0:
                    if state[0] < len(dls) and dls[state[0]][0] <= t + horizon:
                        dls[state[0]][1]()
                        state[0] += 1
                    elif state[1] < len(bgs):
                        bgs[state[1]][1]()
                        state[1] += 1
                    else:
                        break
                    k -= 1

            def drain():
                while state[0] < len(dls):
                    dls[state[0]][1]()
                    state[0] += 1
                while state[1] < len(bgs):
                    bgs[state[1]][1]()
                    state[1] += 1

            return stepper, drain

        # ---- attention for one (batch, sq-block); head pair concurrent
        # via PE row groups. One 1024-wide exp per key tile.
        def attention(b, s, nblk, sched=(), last=False):
            sq0 = s * SQB
            otp0 = otpp.tile([DH + 1, SQB], F32, tag="otp")
            otp1 = otpp.tile([DH + 1, SQB], F32, tag="otp")

            def scores_exp(t):
                scp = sc.tile([128, 2, SQB], F32, tag="scores")
                nc.tensor.matmul(
                    scp[:, 0, :],
                    kt_sb[0:64, b, t * 128 : (t + 1) * 128],
                    qt_sb[0:64, b, sq0 : sq0 + SQB],
                    start=True,
                    stop=True,
                )
                nc.tensor.matmul(
                    scp[:, 1, :],
                    kt_sb[64:128, b, t * 128 : (t + 1) * 128],
                    qt_sb[64:128, b, sq0 : sq0 + SQB],
                    start=True,
                    stop=True,
                )
                pt = ptpool.tile([128, 2, SQB], BF16, tag="pt")
                nc.scalar.activation(
                    pt[:], scp[:], Exp, bias=mb_sb[:, b, t : t + 1], scale=1.0
                )
                return pt

            def av(t, pt):
                nc.tensor.matmul(
                    otp0[:],
                    v_sb[:, b, t, 0, :],
                    pt[:, 0, :],
                    start=(t == 0),
                    stop=(t == nblk - 1),
                )
                nc.tensor.matmul(
                    otp1[:],
                    v_sb[:, b, t, 1, :],
                    pt[:, 1, :],
                    start=(t == 0),
                    stop=(t == nblk - 1),
                )

            stepper, drain = make_stepper(nblk, sched)
            # software pipeline, 2-deep: scores/exp(t+2) emitted FIRST in
            # each iteration (ScalarE never waits on fillers), fillers
            # between, AV(t) last (its exp finished an iteration ago).
            pipe = [scores_exp(0)]
            if nblk > 1:
                pipe.append(scores_exp(1))
            for t in range(nblk):
                stepper(t)
                if t + 2 < nblk:
                    pipe.append(scores_exp(t + 2))
                av(t, pipe.pop(0))
            drain()

            # Copy raw AV psum to SBUF immediately (frees the otp banks
            # ~1.5us after the last AV so the next block's AV never waits),
            # then the whole normalize chain runs off-PSUM, off-PE. The
            # last block skips the staging (shorter tail, no next block).
            otn0 = smpool.tile([DH + 1, SQB], F32, tag="otn", bufs=3, name="otn0")
            otn1 = smpool.tile([DH + 1, SQB], F32, tag="otn", bufs=3, name="otn1")
            if last:
                nc.scalar.copy(otn0[:], otp0[:])
            else:
                nc.vector.tensor_copy(otn0[:], otp0[:])
            nc.vector.tensor_copy(otn1[:], otp1[:])
            rs = smpool.tile([1, 2, SQB], F32, tag="sm")
            if last:
                nc.scalar.copy(rs[:, 0, :], otn0[DH : DH + 1, :])
            else:
                nc.vector.tensor_copy(rs[:, 0, :], otn0[DH : DH + 1, :])
            nc.vector.tensor_copy(rs[:, 1, :], otn1[DH : DH + 1, :])
            recip = smpool.tile([1, 2, SQB], F32, tag="sm")
            nc.vector.reciprocal_approx_fast(recip[:], rs[:])
            if last:
                bc = pp.tile([128, SQB], F32, tag="pp", name="bc")
                nc.tensor.matmul(
                    bc[0:64, :], ones64[:], recip[:, 0, :], start=True, stop=True
                )
                nc.tensor.matmul(
                    bc[64:128, :], ones64[:], recip[:, 1, :],
                    start=True, stop=True, tile_position=(0, 64),
                )
                nc.vector.tensor_mul(
                    ot_sb[0:64, b, sq0 : sq0 + SQB], otn0[0:DH, :], bc[0:64, :]
                )
                nc.vector.tensor_mul(
                    ot_sb[64:128, b, sq0 : sq0 + SQB], otn1[0:DH, :], bc[64:128, :]
                )
            else:
                bcast = smpool.tile([64, 2, SQB], F32, tag="sm")
                nc.gpsimd.partition_broadcast(bcast[:], recip[:])
                nc.vector.tensor_mul(
                    ot_sb[0:64, b, sq0 : sq0 + SQB], otn0[0:DH, :], bcast[:, 0, :]
                )
                nc.vector.tensor_mul(
                    ot_sb[64:128, b, sq0 : sq0 + SQB], otn1[0:DH, :], bcast[:, 1, :]
                )

        # ================= schedule =================
        nkb0, nkb1 = nkbs
        L0 = nblk0 * 128
        xq0, xk0, xv0 = xalloc()
        d0 = xdram(0)
        vt0 = vtpool.tile([128, S], BF16, tag="vt")

        # Need-ordered startup stream. DMA dispatch costs ~0.6us each on
        # the Sync queue, so weights are interleaved with the x chunks in
        # exactly first-use order, 4-chunk granularity for the first
        # blocks (matches 2-MM chain pieces).
        nc.sync.dma_start(out=wq_sb[:], in_=wq_d[:])
        nc.sync.dma_start(out=xq0[:, 0:4, 0:512], in_=d0[0][:, 0:4, 0:512])
        nc.sync.dma_start(out=xq0[:, 4:8, 0:512], in_=d0[0][:, 4:8, 0:512])

        nc.sync.dma_start(out=bias_sb[:], in_=bias_d[:])
        nc.sync.dma_start(out=wk_sb[:], in_=wk_d[:])
        c1 = min(512, L0)
        nc.sync.dma_start(out=xk0[:, 0:4, 0:c1], in_=d0[1][:, 0:4, 0:c1])
        nc.sync.dma_start(out=xk0[:, 4:8, 0:c1], in_=d0[1][:, 4:8, 0:c1])
        nc.sync.dma_start(out=wv_sb[:], in_=wv_d[:])
        nc.sync.dma_start(out=xv0[:, 0:4, 0:c1], in_=d0[2][:, 0:4, 0:c1])
        nc.sync.dma_start(out=xv0[:, 4:8, 0:c1], in_=d0[2][:, 4:8, 0:c1])
        nc.sync.dma_start(out=id_sb[:], in_=id_d[:])
        nc.sync.dma_start(out=mb_sb[:], in_=mb_d.rearrange("p (b t) -> p b t", b=B))
        if L0 > 512:
            nc.sync.dma_start(out=xk0[:, :, 512:L0], in_=d0[1][:, :, 512:L0])
            nc.sync.dma_start(out=xv0[:, :, 512:L0], in_=d0[2][:, :, 512:L0])
        nc.sync.dma_start(out=xq0[:, :, 512:1024], in_=d0[0][:, :, 512:1024])
        nc.sync.dma_start(out=wo_sb[:], in_=wo_d[:])
        nc.sync.dma_start(out=xq0[:, :, 1024:S], in_=d0[0][:, :, 1024:S])

        # prologue: minimal work for the first exp
        for u in q_units(0, xq0, 0):
            u()
        for u in k_units(0, xk0, 0, nblk0):
            u()
        for u in vt_units(0, xv0, 0, nblk0, vt0):
            u()
        transp_unit(0, vt0, 0)()

        # A(0,0): stream remaining batch-0 k/v chains + transposes with
        # deadlines; spread q(0,1) afterwards.
        sched = []
        for j in range(1, nkb0):
            dl = max(0, 4 * j - 3)
            for u in k_units(0, xk0, j, nblk0):
                sched.append((dl, u))
            for u in vt_units(0, xv0, j, nblk0, vt0):
                sched.append((dl, u))
        for t in range(1, nblk0):
            sched.append((t, transp_unit(0, vt0, t)))
        for u in q_units(0, xq0, 1):
            sched.append((nblk0 + 9, u))
        attention(0, 0, nblk0, sched)

        # A(0,1): q(0,2), q(0,3), outproj(0,0)
        sched = [(nblk0 + 9, u) for u in q_units(0, xq0, 2)]
        sched += [(nblk0 + 9, u) for u in q_units(0, xq0, 3)]
        sched += [(nblk0 + 9, u) for u in outproj_units(0, 0)]
        attention(0, 1, nblk0, sched)

        # batch-1 loads; its projections fill A(0,2)/A(0,3)
        xq1, xk1, xv1 = xalloc()
        load_x1(1, nblk1, xq1, xk1, xv1)
        vt1 = vtpool.tile([128, S], BF16, tag="vt")

        sched = [(nblk0 + 9, u) for u in q_units(1, xq1, 0)]
        for j in range(nkb1):
            sched += [(nblk0 + 9, u) for u in k_units(1, xk1, j, nblk1)]
            sched += [(nblk0 + 9, u) for u in vt_units(1, xv1, j, nblk1, vt1)]
        attention(0, 2, nblk0, sched)

        sched = [(nblk0 + 9, transp_unit(1, vt1, t)) for t in range(nblk1)]
        sched += [(nblk0 + 9, u) for u in q_units(1, xq1, 1)]
        sched += [(nblk0 + 9, u) for u in outproj_units(0, 1)]
        attention(0, 3, nblk0, sched)

        sched = [(nblk1 + 9, u) for u in q_units(1, xq1, 2)]
        sched += [(nblk1 + 9, u) for u in q_units(1, xq1, 3)]
        sched += [(nblk1 + 9, u) for u in outproj_units(0, 2)]
        attention(1, 0, nblk1, sched)

        sched = [(nblk1 + 9, u) for u in outproj_units(0, 3)]
        sched += [(nblk1 + 9, u) for u in outproj_units(1, 0)]
        attention(1, 1, nblk1, sched)

        sched = [(nblk1 + 9, u) for u in outproj_units(1, 1)]
        attention(1, 2, nblk1, sched)

        sched = [(max(0, nblk1 - 5), u) for u in outproj_units(1, 2)]
        attention(1, 3, nblk1, sched, last=True)

        for u in outproj_units(1, 3, tail=True):
            u()

    nc.compile()
    return nc


def _chunk_rows(w: np.ndarray, nchunk: int) -> np.ndarray:
    """[nchunk*128, C] -> [128, nchunk*C] with chunk-major columns."""
    c = w.shape[1]
    return np.ascontiguousarray(
        w.reshape(nchunk, 128, c).transpose(1, 0, 2).reshape(128, nchunk * c)
    )


def make_inmaps(inputs: dict):
    xq = np.asarray(inputs["xq"], np.float32)
    xk = np.asarray(inputs["xk"], np.float32)
    xv = np.asarray(inputs["xv"], np.float32)
    wq = np.asarray(inputs["wq"], np.float32)
    bq = np.asarray(inputs["bq"], np.float32)
    wk = np.asarray(inputs["wk"], np.float32)
    bk = np.asarray(inputs["bk"], np.float32)
    wv = np.asarray(inputs["wv"], np.float32)
    bv = np.asarray(inputs["bv"], np.float32)
    wo = np.asarray(inputs["wo"], np.float32)
    valid_lens = np.asarray(inputs["valid_lens"], np.int64)

    nblks = tuple(
        int(min(NST, max(1, -(-int(valid_lens[b]) // 128)))) for b in range(B)
    )

    # shared per-batch transposed activations (bf16)
    xts = {}
    for b in range(B):
        for n, a in (("q", xq), ("k", xk), ("v", xv)):
            xts[f"x{n}t{b}"] = np.ascontiguousarray(a[b].T).astype(npbf16)

    # mask bias columns [128, B*NST]
    mbs = []
    for b in range(B):
        bias = np.where(np.arange(S) < int(valid_lens[b]), 0.0, MASK_BIAS).astype(
            np.float32
        )
        mbs.append(bias.reshape(NST, 128).T)
    mb = np.ascontiguousarray(np.concatenate(mbs, axis=1))
    ident = np.eye(128, dtype=npbf16)

    in_maps = []
    for c in range(NCORES):
        sl = slice(c * PW, (c + 1) * PW)
        bias3 = np.stack([bq[sl] * SCALE, bk[sl], bv[sl]], axis=1).astype(np.float32)
        in_maps.append(
            {
                **xts,
                "wq": _chunk_rows(wq[:, sl] * SCALE, NKC).astype(npbf16),
                "wk": _chunk_rows(wk[:, sl], NKC).astype(npbf16),
                "wv": _chunk_rows(wv[:, sl], NKC).astype(npbf16),
                "wo": np.ascontiguousarray(wo[sl, :]).astype(npbf16),
                "bias": np.ascontiguousarray(bias3),
                "mb": mb,
                "ident": ident,
            }
        )
    return in_maps, nblks


def assemble(results, inputs) -> np.ndarray:
    bo = np.asarray(inputs["bo"], np.float32)
    out = np.zeros((B, S, D), np.float32)
    for c in range(NCORES):
        part = np.asarray(results[c]["outt"], np.float32).reshape(B, D, S)
        for b in range(B):
            out[b] += part[b].T
    out += bo[None, None, :]
    return out


def kernel(**inputs) -> np.ndarray:
    in_maps, nblks = make_inmaps(inputs)
    nc = build_nc(nblks)
    res = run_bass_kernel_spmd(nc, in_maps, core_ids=list(range(NCORES)))
    return assemble(res.results, inputs)


if __name__ == "__main__":
    import reference

    inputs = reference.setup_inputs()
    out = kernel(**{k: np.asarray(v) for k, v in inputs.items()})
    exp = np.asarray(reference.reference(**inputs))
    err = np.linalg.norm(out - exp) / np.linalg.norm(exp)
    print("Relative error:", err)


# revision 25
# speedup vs baseline: 1.1936x; 1.1936x over previous
"""Multi-head attention (B=2,S=2048,D=1024,H=16) on 8 TRN2 NeuronCores.

Sharding: core c handles head-PAIR c (heads 2c, 2c+1) of BOTH batches
(tensor parallel over heads; both batches per core so per-batch key-tile
counts need no SPMD padding). wq/wk/wv split column-wise by pair, wo
row-wise. Each core computes partial output projections [D,S] per batch;
the host sums the 8 partials, transposes, adds bo.

v2 design (vs v1 baseline):
  - vT computed like kT (wv chunks stationary, 512-wide moving) then
    PE-transposed per 128-tile into AV layout -> kills the 198 tiny
    LDW-bound matmuls of v1.
  - q/k/v biases folded into the PSUM->SBUF copies (tensor_scalar_add
    with per-partition bias column) -> no bias matmuls.
  - Fully interleaved emission: ScalarE exp (the 117us wall: 88 ACTs)
    starts ~10us in and streams continuously; all proj/outproj PE work
    is emitted in small "filler" units inside attention t-loops so the
    PE works during exp waits instead of front-loading projections.
  - Finer, need-ordered input DMAs; merged output DMAs.

Per-core device layout ("T" = [feature, seq]):
  qT[b] = (wq_p^T @ xq_b^T)*0.125 + bq/8   [128, S]
  kT[b] =  wk_p^T @ xk_b^T + bk            [128, S]  (valid cols only)
  vT[b] =  wv_p^T @ xv_b^T + bv            [128, S]  -> transpose 128-tiles
  v[b]  [sk, b, t, head, 65] with ones column for softmax denominators
  per (b, sq-block, key-tile t):
    scoresT(hh) = kT_h[:,t]^T-stat @ qT_h    [128 sk, 512 sq] psum (pair
      co-runs on PE row groups 0-63 / 64-127)
    pT = exp(scoresT + mask_bias[b][t])      one 1024-wide ACT
    oT_ext(hh) += [v_h[t] | 1]^T-stat @ pT(hh)   [65, 512] psum
  oT = oT_ext[0:64] * bcast(1/rowsum);  outT[b] += wo_p^T @ oT
"""

import sys

if "/opt/trn_rl_repo" not in sys.path:
    sys.path.insert(0, "/opt/trn_rl_repo")

from contextlib import ExitStack

import numpy as np
import ml_dtypes

from concourse import bass, bacc, mybir
from concourse import tile
from concourse.bass_utils import run_bass_kernel_spmd

BF16 = mybir.dt.bfloat16
F32 = mybir.dt.float32
npbf16 = ml_dtypes.bfloat16

B, S, D, H, DH = 2, 2048, 1024, 16, 64
NCORES = 8
PW = 2 * DH  # 128: head-pair width = per-core projection width
NKC = D // 128  # 8 contraction chunks for projections
NST = S // 128  # 16 key tiles
SQB = 512
NSQB = S // SQB  # 4
NDT = D // 128  # 8 output row-tiles
SCALE = 1.0 / 8.0  # 1/sqrt(DH)
MASK_BIAS = -30000.0


def build_nc(nblks) -> bass.Bass:
    nblk0, nblk1 = nblks
    nkbs = tuple(-(-nb * 128 // SQB) for nb in nblks)  # valid 512-col blocks
    nc = bacc.Bacc()

    x_d = []
    for b in range(B):
        x_d.append(
            tuple(
                nc.declare_dram_parameter(f"x{n}t{b}", [D, S], BF16, isOutput=False)
                for n in "qkv"
            )
        )
    wq_d = nc.declare_dram_parameter("wq", [128, NKC * PW], BF16, isOutput=False)
    wk_d = nc.declare_dram_parameter("wk", [128, NKC * PW], BF16, isOutput=False)
    wv_d = nc.declare_dram_parameter("wv", [128, NKC * PW], BF16, isOutput=False)
    wo_d = nc.declare_dram_parameter("wo", [128, D], BF16, isOutput=False)
    bias_d = nc.declare_dram_parameter("bias", [128, 3], F32, isOutput=False)
    mb_d = nc.declare_dram_parameter("mb", [128, B * NST], F32, isOutput=False)
    id_d = nc.declare_dram_parameter("ident", [128, 128], BF16, isOutput=False)
    out_d = nc.declare_dram_parameter("outt", [B * D, S], BF16, isOutput=True)
    outv = out_d.rearrange("(x p) s -> p x s", p=128)  # [128, B*NDT, S]

    Exp = mybir.ActivationFunctionType.Exp

    with tile.TileContext(nc) as tc, ExitStack() as ctx:
        cpool = ctx.enter_context(tc.tile_pool(name="consts", bufs=1))
        xpool = ctx.enter_context(tc.tile_pool(name="xin", bufs=4))
        qkpool = ctx.enter_context(tc.tile_pool(name="qk", bufs=1))
        vtpool = ctx.enter_context(tc.tile_pool(name="vt", bufs=1))
        vpool = ctx.enter_context(tc.tile_pool(name="vsb", bufs=1))
        opool = ctx.enter_context(tc.tile_pool(name="osb", bufs=1))
        ptpool = ctx.enter_context(tc.tile_pool(name="ptp", bufs=3))
        smpool = ctx.enter_context(tc.tile_pool(name="small", bufs=3))
        outpool = ctx.enter_context(tc.tile_pool(name="outsb", bufs=2))
        pp = ctx.enter_context(tc.tile_pool(name="pp", bufs=2, space="PSUM"))
        sc = ctx.enter_context(tc.tile_pool(name="sc", bufs=2, space="PSUM"))
        otpp = ctx.enter_context(tc.tile_pool(name="otp", bufs=2, space="PSUM"))

        # ---- constants / weights ----
        wq_sb = cpool.tile([128, NKC * PW], BF16, tag="wq")
        wk_sb = cpool.tile([128, NKC * PW], BF16, tag="wk")
        wv_sb = cpool.tile([128, NKC * PW], BF16, tag="wv")
        wo_sb = cpool.tile([128, D], BF16, tag="wo")
        bias_sb = cpool.tile([128, 3], F32, tag="bias")
        mb_sb = cpool.tile([128, B, NST], F32, tag="mb")
        id_sb = cpool.tile([128, 128], BF16, tag="ident")

        qt_sb = qkpool.tile([128, B, S], BF16, tag="qt")
        kt_sb = qkpool.tile([128, B, S], BF16, tag="kt")
        # v with a trailing ones column per head (partition reads must be
        # 32-aligned, so the denominator sits at psum partition 64):
        # [sk-part, b, tile, head, dh+1]
        v_sb = vpool.tile([128, B, NST, 2, DH + 1], BF16, tag="v")
        nc.gpsimd.memset(v_sb[:, :, :, :, DH : DH + 1], 1.0)
        ot_sb = opool.tile([128, B, S], BF16, tag="ot")
        ones64 = cpool.tile([1, DH], F32, tag="ones64")
        nc.gpsimd.memset(ones64[:], 1.0)

        def xalloc():
            xq = xpool.tile([128, NKC, S], BF16, tag="xt", name="xq")
            xk = xpool.tile([128, NKC, S], BF16, tag="xt", name="xk")
            xv = xpool.tile([128, NKC, S], BF16, tag="xt", name="xv")
            return xq, xk, xv

        def xdram(b):
            return [x_d[b][i].rearrange("(c p) s -> p c s", p=128) for i in range(3)]

        def load_x1(b, nblk, xq, xk, xv):
            """Background batch: few big dispatches, need-ordered."""
            L = nblk * 128
            drams = xdram(b)
            nc.sync.dma_start(out=xq[:, :, 0:1024], in_=drams[0][:, :, 0:1024])
            for t, dram in ((xk, drams[1]), (xv, drams[2])):
                nc.sync.dma_start(out=t[:, :, 0:L], in_=dram[:, :, 0:L])
            nc.sync.dma_start(out=xq[:, :, 1024:S], in_=drams[0][:, :, 1024:S])

        # ---- projection chains: contraction over 8 chunks, bias folded
        # into the DVE copy. Emitted as pieces (2 MMs each) so they can be
        # spread into attention loops as PE filler.
        def chain_units(dst, w_sb, xt, c0, c1, bcol):
            n = c1 - c0
            box = []

            def piece(i):
                if i == 0:
                    box.append(pp.tile([128, SQB], F32, tag="pp", name="chps"))
                ps = box[0]
                for c in range(2 * i, 2 * i + 2):
                    nc.tensor.matmul(
                        ps[:, 0:n],
                        w_sb[:, c * PW : (c + 1) * PW],
                        xt[:, c, c0:c1],
                        start=(c == 0),
                        stop=(c == NKC - 1),
                    )
                if i == 3:
                    nc.vector.tensor_scalar_add(dst[:, c0:c1], ps[:, 0:n], bcol)

            return [(lambda i=i: piece(i)) for i in range(4)]

        def q_units(b, xt, s):
            return chain_units(
                qt_sb[:, b, :], wq_sb, xt, s * SQB, (s + 1) * SQB, bias_sb[:, 0:1]
            )

        def k_units(b, xt, j, nblk):
            c1 = min((j + 1) * SQB, nblk * 128)
            return chain_units(kt_sb[:, b, :], wk_sb, xt, j * SQB, c1, bias_sb[:, 1:2])

        def vt_units(b, xt, j, nblk, vt):
            c1 = min((j + 1) * SQB, nblk * 128)
            return chain_units(vt, wv_sb, xt, j * SQB, c1, bias_sb[:, 2:3])

        def transp_unit(b, vt, t):
            def u():
                ps = pp.tile([128, 128], BF16, tag="pp", name="trps")
                nc.tensor.transpose(
                    ps[:], vt[:, t * 128 : (t + 1) * 128], id_sb[:]
                )
                nc.vector.tensor_copy(
                    v_sb[:, b, t, :, 0:DH],
                    ps.rearrange("p (h d) -> p h d", h=2),
                )

            return u

        # ---- fused partial output projection, one unit per 128-row tile;
        # DMA per 4-tile half.
        def outproj_units(b, s, tail=False):
            sq0 = s * SQB
            halves = []

            def mk(dt):
                def u():
                    if dt % 4 == 0:
                        halves.append(outpool.tile([128, 4, SQB], BF16, tag="osb", name="osb"))
                    osb = halves[-1]
                    pso = pp.tile([128, SQB], F32, tag="pp", name="pso")
                    nc.tensor.matmul(
                        pso[:],
                        wo_sb[:, dt * 128 : (dt + 1) * 128],
                        ot_sb[:, b, sq0 : sq0 + SQB],
                        start=True,
                        stop=True,
                    )
                    if tail and dt % 2 == 1:
                        nc.scalar.copy(osb[:, dt % 4, :], pso[:])
                    else:
                        nc.vector.tensor_copy(osb[:, dt % 4, :], pso[:])
                    if dt % 4 == 3:
                        h = dt // 4
                        nc.sync.dma_start(
                            out=outv[
                                :, b * NDT + 4 * h : b * NDT + 4 * h + 4,
                                sq0 : sq0 + SQB,
                            ],
                            in_=osb[:],
                        )

                return u

            return [mk(dt) for dt in range(NDT)]

        # ---- filler machinery: (deadline, unit) lists consumed inside
        # attention t-loops. deadline d => must be emitted by stepper call
        # at t-step d (before scores(d+1) / av(d)). Chains are atomic
        # (consecutive units) but may interleave with other pp users only
        # pairwise (pp bufs=2).
        def make_stepper(nsteps, sched, horizon=6):
            dls = sorted([u for u in sched if u[0] <= nsteps], key=lambda x: x[0])
            bgs = [u for u in sched if u[0] > nsteps]
            state = [0, 0]

            def stepper(t):
                while state[0] < len(dls) and dls[state[0]][0] <= t:
                    dls[state[0]][1]()
                    state[0] += 1
                left = (len(dls) - state[0]) + (len(bgs) - state[1])
                k = -(-left // max(1, nsteps - t))
                while k > 0:
                    if state[0] < len(dls) and dls[state[0]][0] <= t + horizon:
                        dls[state[0]][1]()
                        state[0] += 1
                    elif state[1] < len(bgs):
                        bgs[state[1]][1]()
                        state[1] += 1
                    else:
                        break
                    k -= 1

            def drain():
                while state[0] < len(dls):
                    dls[state[0]][1]()
                    state[0] += 1
                while state[1] < len(bgs):
                    bgs[state[1]][1]()
                    state[1] += 1

            return stepper, drain

        # ---- attention for one (batch, sq-block); head pair concurrent
        # via PE row groups. One 1024-wide exp per key tile.
        def attention(b, s, nblk, sched=(), last=False):
            sq0 = s * SQB
            otp0 = otpp.tile([DH + 1, SQB], F32, tag="otp")
            otp1 = otpp.tile([DH + 1, SQB], F32, tag="otp")

            def scores_exp(t):
                scp = sc.tile([128, 2, SQB], F32, tag="scores")
                nc.tensor.matmul(
                    scp[:, 0, :],
                    kt_sb[0:64, b, t * 128 : (t + 1) * 128],
                    qt_sb[0:64, b, sq0 : sq0 + SQB],
                    start=True,
                    stop=True,
                )
                nc.tensor.matmul(
                    scp[:, 1, :],
                    kt_sb[64:128, b, t * 128 : (t + 1) * 128],
                    qt_sb[64:128, b, sq0 : sq0 + SQB],
                    start=True,
                    stop=True,
                )
                pt = ptpool.tile([128, 2, SQB], BF16, tag="pt")
                nc.scalar.activation(
                    pt[:], scp[:], Exp, bias=mb_sb[:, b, t : t + 1], scale=1.0
                )
                return pt

            def av(t, pt):
                nc.tensor.matmul(
                    otp0[:],
                    v_sb[:, b, t, 0, :],
                    pt[:, 0, :],
                    start=(t == 0),
                    stop=(t == nblk - 1),
                )
                nc.tensor.matmul(
                    otp1[:],
                    v_sb[:, b, t, 1, :],
                    pt[:, 1, :],
                    start=(t == 0),
                    stop=(t == nblk - 1),
                )

            stepper, drain = make_stepper(nblk, sched)
            # software pipeline, 2-deep: scores/exp(t+2) emitted FIRST in
            # each iteration (ScalarE never waits on fillers), fillers
            # between, AV(t) last (its exp finished an iteration ago).
            pipe = [scores_exp(0)]
            if nblk > 1:
                pipe.append(scores_exp(1))
            for t in range(nblk):
                stepper(t)
                if t + 2 < nblk:
                    pipe.append(scores_exp(t + 2))
                av(t, pipe.pop(0))
            drain()

            # Copy raw AV psum to SBUF immediately (frees the otp banks
            # ~1.5us after the last AV so the next block's AV never waits),
            # then the whole normalize chain runs off-PSUM, off-PE. The
            # last block skips the staging (shorter tail, no next block).
            otn0 = smpool.tile([DH + 1, SQB], F32, tag="otn", bufs=4, name="otn0")
            otn1 = smpool.tile([DH + 1, SQB], F32, tag="otn", bufs=4, name="otn1")
            if last:
                nc.scalar.copy(otn0[:], otp0[:])
            else:
                nc.vector.tensor_copy(otn0[:], otp0[:])
            nc.vector.tensor_copy(otn1[:], otp1[:])
            rs = smpool.tile([1, 2, SQB], F32, tag="sm")
            if last:
                nc.scalar.copy(rs[:, 0, :], otn0[DH : DH + 1, :])
            else:
                nc.vector.tensor_copy(rs[:, 0, :], otn0[DH : DH + 1, :])
            nc.vector.tensor_copy(rs[:, 1, :], otn1[DH : DH + 1, :])
            recip = smpool.tile([1, 2, SQB], F32, tag="sm")
            nc.vector.reciprocal_approx_fast(recip[:], rs[:])
            if last:
                bc = pp.tile([128, SQB], F32, tag="pp", name="bc")
                nc.tensor.matmul(
                    bc[0:64, :], ones64[:], recip[:, 0, :], start=True, stop=True
                )
                nc.tensor.matmul(
                    bc[64:128, :], ones64[:], recip[:, 1, :],
                    start=True, stop=True, tile_position=(0, 64),
                )
                nc.vector.tensor_mul(
                    ot_sb[0:64, b, sq0 : sq0 + SQB], otn0[0:DH, :], bc[0:64, :]
                )
                nc.vector.tensor_mul(
                    ot_sb[64:128, b, sq0 : sq0 + SQB], otn1[0:DH, :], bc[64:128, :]
                )
            else:
                bcast = smpool.tile([64, 2, SQB], F32, tag="sm")
                nc.gpsimd.partition_broadcast(bcast[:], recip[:])
                nc.vector.tensor_mul(
                    ot_sb[0:64, b, sq0 : sq0 + SQB], otn0[0:DH, :], bcast[:, 0, :]
                )
                nc.vector.tensor_mul(
                    ot_sb[64:128, b, sq0 : sq0 + SQB], otn1[0:DH, :], bcast[:, 1, :]
                )

        # ================= schedule =================
        nkb0, nkb1 = nkbs
        L0 = nblk0 * 128
        xq0, xk0, xv0 = xalloc()
        d0 = xdram(0)
        vt0 = vtpool.tile([128, S], BF16, tag="vt")

        # Need-ordered startup stream. DMA dispatch costs ~0.6us each on
        # the Sync queue, so weights are interleaved with the x chunks in
        # exactly first-use order, 4-chunk granularity for the first
        # blocks (matches 2-MM chain pieces).
        nc.sync.dma_start(out=wq_sb[:], in_=wq_d[:])
        nc.sync.dma_start(out=xq0[:, 0:4, 0:512], in_=d0[0][:, 0:4, 0:512])
        nc.sync.dma_start(out=xq0[:, 4:8, 0:512], in_=d0[0][:, 4:8, 0:512])

        nc.sync.dma_start(out=bias_sb[:], in_=bias_d[:])
        nc.sync.dma_start(out=wk_sb[:], in_=wk_d[:])
        c1 = min(512, L0)
        nc.sync.dma_start(out=xk0[:, 0:4, 0:c1], in_=d0[1][:, 0:4, 0:c1])
        nc.sync.dma_start(out=xk0[:, 4:8, 0:c1], in_=d0[1][:, 4:8, 0:c1])
        nc.sync.dma_start(out=wv_sb[:], in_=wv_d[:])
        nc.sync.dma_start(out=xv0[:, 0:4, 0:c1], in_=d0[2][:, 0:4, 0:c1])
        nc.sync.dma_start(out=xv0[:, 4:8, 0:c1], in_=d0[2][:, 4:8, 0:c1])
        nc.sync.dma_start(out=id_sb[:], in_=id_d[:])
        nc.sync.dma_start(out=mb_sb[:], in_=mb_d.rearrange("p (b t) -> p b t", b=B))
        if L0 > 512:
            nc.sync.dma_start(out=xk0[:, :, 512:L0], in_=d0[1][:, :, 512:L0])
            nc.sync.dma_start(out=xv0[:, :, 512:L0], in_=d0[2][:, :, 512:L0])
        nc.sync.dma_start(out=xq0[:, :, 512:1024], in_=d0[0][:, :, 512:1024])
        nc.sync.dma_start(out=wo_sb[:], in_=wo_d[:])
        nc.sync.dma_start(out=xq0[:, :, 1024:S], in_=d0[0][:, :, 1024:S])

        # prologue: minimal work for the first exp
        for u in q_units(0, xq0, 0):
            u()
        for u in k_units(0, xk0, 0, nblk0):
            u()
        for u in vt_units(0, xv0, 0, nblk0, vt0):
            u()
        transp_unit(0, vt0, 0)()

        # A(0,0): stream remaining batch-0 k/v chains + transposes with
        # deadlines; spread q(0,1) afterwards.
        sched = []
        for j in range(1, nkb0):
            dl = max(0, 4 * j - 3)
            for u in k_units(0, xk0, j, nblk0):
                sched.append((dl, u))
            for u in vt_units(0, xv0, j, nblk0, vt0):
                sched.append((dl, u))
        for t in range(1, nblk0):
            sched.append((t, transp_unit(0, vt0, t)))
        for u in q_units(0, xq0, 1):
            sched.append((nblk0 + 9, u))
        attention(0, 0, nblk0, sched)

        # A(0,1): q(0,2), q(0,3), outproj(0,0)
        sched = [(nblk0 + 9, u) for u in q_units(0, xq0, 2)]
        sched += [(nblk0 + 9, u) for u in q_units(0, xq0, 3)]
        sched += [(nblk0 + 9, u) for u in outproj_units(0, 0)]
        attention(0, 1, nblk0, sched)

        # batch-1 loads; its projections fill A(0,2)/A(0,3)
        xq1, xk1, xv1 = xalloc()
        load_x1(1, nblk1, xq1, xk1, xv1)
        vt1 = vtpool.tile([128, S], BF16, tag="vt")

        sched = [(nblk0 + 9, u) for u in q_units(1, xq1, 0)]
        for j in range(nkb1):
            sched += [(nblk0 + 9, u) for u in k_units(1, xk1, j, nblk1)]
            sched += [(nblk0 + 9, u) for u in vt_units(1, xv1, j, nblk1, vt1)]
        attention(0, 2, nblk0, sched)

        sched = [(nblk0 + 9, transp_unit(1, vt1, t)) for t in range(nblk1)]
        sched += [(nblk0 + 9, u) for u in q_units(1, xq1, 1)]
        sched += [(nblk0 + 9, u) for u in outproj_units(0, 1)]
        attention(0, 3, nblk0, sched)

        sched = [(nblk1 + 9, u) for u in q_units(1, xq1, 2)]
        sched += [(nblk1 + 9, u) for u in q_units(1, xq1, 3)]
        sched += [(nblk1 + 9, u) for u in outproj_units(0, 2)]
        attention(1, 0, nblk1, sched)

        sched = [(nblk1 + 9, u) for u in outproj_units(0, 3)]
        sched += [(nblk1 + 9, u) for u in outproj_units(1, 0)]
        attention(1, 1, nblk1, sched)

        sched = [(nblk1 + 9, u) for u in outproj_units(1, 1)]
        attention(1, 2, nblk1, sched)

        sched = [(max(0, nblk1 - 5), u) for u in outproj_units(1, 2)]
        attention(1, 3, nblk1, sched, last=True)

        for u in outproj_units(1, 3, tail=True):
            u()

    nc.compile()
    return nc


def _chunk_rows(w: np.ndarray, nchunk: int) -> np.ndarray:
    """[nchunk*128, C] -> [128, nchunk*C] with chunk-major columns."""
    c = w.shape[1]
    return np.ascontiguousarray(
        w.reshape(nchunk, 128, c).transpose(1, 0, 2).reshape(128, nchunk * c)
    )


def make_inmaps(inputs: dict):
    xq = np.asarray(inputs["xq"], np.float32)
    xk = np.asarray(inputs["xk"], np.float32)
    xv = np.asarray(inputs["xv"], np.float32)
    wq = np.asarray(inputs["wq"], np.float32)
    bq = np.asarray(inputs["bq"], np.float32)
    wk = np.asarray(inputs["wk"], np.float32)
    bk = np.asarray(inputs["bk"], np.float32)
    wv = np.asarray(inputs["wv"], np.float32)
    bv = np.asarray(inputs["bv"], np.float32)
    wo = np.asarray(inputs["wo"], np.float32)
    valid_lens = np.asarray(inputs["valid_lens"], np.int64)

    nblks = tuple(
        int(min(NST, max(1, -(-int(valid_lens[b]) // 128)))) for b in range(B)
    )

    # shared per-batch transposed activations (bf16)
    xts = {}
    for b in range(B):
        for n, a in (("q", xq), ("k", xk), ("v", xv)):
            xts[f"x{n}t{b}"] = np.ascontiguousarray(a[b].T).astype(npbf16)

    # mask bias columns [128, B*NST]
    mbs = []
    for b in range(B):
        bias = np.where(np.arange(S) < int(valid_lens[b]), 0.0, MASK_BIAS).astype(
            np.float32
        )
        mbs.append(bias.reshape(NST, 128).T)
    mb = np.ascontiguousarray(np.concatenate(mbs, axis=1))
    ident = np.eye(128, dtype=npbf16)

    in_maps = []
    for c in range(NCORES):
        sl = slice(c * PW, (c + 1) * PW)
        bias3 = np.stack([bq[sl] * SCALE, bk[sl], bv[sl]], axis=1).astype(np.float32)
        in_maps.append(
            {
                **xts,
                "wq": _chunk_rows(wq[:, sl] * SCALE, NKC).astype(npbf16),
                "wk": _chunk_rows(wk[:, sl], NKC).astype(npbf16),
                "wv": _chunk_rows(wv[:, sl], NKC).astype(npbf16),
                "wo": np.ascontiguousarray(wo[sl, :]).astype(npbf16),
                "bias": np.ascontiguousarray(bias3),
                "mb": mb,
                "ident": ident,
            }
        )
    return in_maps, nblks


def assemble(results, inputs) -> np.ndarray:
    bo = np.asarray(inputs["bo"], np.float32)
    out = np.zeros((B, S, D), np.float32)
    for c in range(NCORES):
        part = np.asarray(results[c]["outt"], np.float32).reshape(B, D, S)
        for b in range(B):
            out[b] += part[b].T
    out += bo[None, None, :]
    return out


def kernel(**inputs) -> np.ndarray:
    in_maps, nblks = make_inmaps(inputs)
    nc = build_nc(nblks)
    res = run_bass_kernel_spmd(nc, in_maps, core_ids=list(range(NCORES)))
    return assemble(res.results, inputs)


if __name__ == "__main__":
    import reference

    inputs = reference.setup_inputs()
    out = kernel(**{k: np.asarray(v) for k, v in inputs.items()})
    exp = np.asarray(reference.reference(**inputs))
    err = np.linalg.norm(out - exp) / np.linalg.norm(exp)
    print("Relative error:", err)
